# revision 15
# baseline (speedup 1.0000x reference)
"""Trainium2 Bass kernel for nn_BERT_tensor (8-layer BERT with tensor-network heads).

Strategy (v3):
  - Data-parallel over batch: 32 seqs -> 4 seqs (800 tokens) per core x 8 cores.
  - Host folds the MPO tensor-network contraction (A1..A4) into a dense
    [256 -> 1024] weight per (layer, q/k/v); QKV is one dense matmul.
  - Everything stays DIM-MAJOR ([dim, token]); zero PE transposes:
    * attention computes scores TRANSPOSED ([kpos, qpos]); softmax
      max-subtraction replaced by a constant shift (scores bounded ~|14|);
      denominator via ones-matmul; division via reciprocal_approx_fast;
      normalization fused into ctx evacuation.
    * LayerNorm dim-major: stats via ones-matmuls over the partition axis;
      rstd = exp(-0.5*ln(var+eps)) on ScalarE (no Sqrt table set, no slow
      DVE reciprocal); per-token scale/offset broadcast via rank-1
      outer-product matmuls with the LN gain as the stationary operand.
  - fp16 matmul inputs everywhere (fp32 PSUM accumulation).
  - Single 1-bank PSUM pool (8 slots), 400-col chunks for fine-grained
    pipelining to keep TensorE dense (HAM warm).
"""
import numpy as np
from contextlib import ExitStack

import concourse.bass as bass
import concourse.bacc as bacc
import concourse.tile as tile
import concourse.mybir as mybir
from concourse.bass_utils import run_bass_kernel_spmd

dt = mybir.dt
AF = mybir.ActivationFunctionType
ALU = mybir.AluOpType

# problem constants (hardcoded per contract)
B, S, D = 32, 200, 256
H, DFF, VOCAB, L, TD = 6, 1024, 3500, 8, 2
N_CORES = 8
BS = B // N_CORES            # 4 seqs per core
T = BS * S                   # 800 tokens per core
KT = D // 128                # 2 k-tiles over emb dim
NQK = (2 * H * D) // 128     # 24 m-tiles over Q|K outdim (3072)
NCTX = (H * D) // 128        # 12 tiles over ctx dim (1536)
NMID = DFF // 128            # 8 tiles over ffn hidden
MMCH = [(0, 400), (400, 400)]       # N chunks of T (each fits one PSUM bank)
SEQ_TILES = [(0, 128), (128, 72)]   # per-seq kpos tiles
EPS = 1e-6
CSHIFT = 6.0                 # softmax constant shift (max |score| ~ 14)

import os
L_RUN = int(os.environ.get("BERT_L_RUN", str(L)))
DT_MM = dt.float16
NP_MM = np.float16

_CACHE = {}


def _patch_act_tables():
    """Force every activation function to resolve to the
    natural_log_exp_and_others table set (it contains exp/ln/square/
    identity/copy/relu — everything this kernel uses), so exactly one
    ACT_TABLE_LOAD is emitted instead of one per exp<->ln alternation.
    Set names and their act_info.json indices are preserved."""
    import concourse.hw_specs as hw_specs
    import concourse.bacc as bacc_mod
    if getattr(bacc_mod, "_act_tables_patched", False):
        return
    orig = hw_specs.get_activation_tables

    def only_nle(arch):
        t = orig(arch)
        return {k: (v if k == "natural_log_exp_and_others" else set())
                for k, v in t.items()}

    bacc_mod.get_activation_tables = only_nle
    bacc_mod._act_tables_patched = True


def _build_program():
    _patch_act_tables()
    nc = bacc.Bacc("TRN2", target_bir_lowering=False, debug=False,
                   num_devices=N_CORES)

    f32 = dt.float32
    inp = {}

    def din(name, shape, dty):
        inp[name] = nc.dram_tensor(name, list(shape), dty, kind="ExternalInput").ap()
        return inp[name]

    h0_d = din("h0", [KT, 128, T], DT_MM)
    maskc_d = din("maskc", [128, BS * KT], f32)        # exp bias cols (mask - C)
    wqk_d = din("wqk", [L, 128, KT, 2 * H * D], DT_MM)
    bqk_d = din("bqk", [L, 128, NQK], f32)
    wv_d = din("wv", [L, 128, KT, H * D], DT_MM)
    ow_d = din("ow", [L, 128, NCTX, D], DT_MM)
    obe_d = din("obe", [L, 128, KT], f32)
    ff1_d = din("ff1", [L, 128, KT, DFF], DT_MM)
    f1b_d = din("f1b", [L, 128, NMID], f32)
    ff2_d = din("ff2", [L, 128, NMID, D], DT_MM)
    f2b_d = din("f2b", [L, 128, KT], f32)
    bcol_d = din("bcol", [L, 128, 2 * KT], f32)        # b1|b2 cols
    grow_d = din("grow", [L, 1, 2 * KT * 128], DT_MM)  # g1|g2 rows (fp16)
    out_d = nc.dram_tensor("out", [KT, 128, T], f32, kind="ExternalOutput").ap()

    with tile.TileContext(nc) as tc:
        with ExitStack() as ctx:
            cpool = ctx.enter_context(tc.tile_pool(name="const", bufs=1))
            wpool = ctx.enter_context(tc.tile_pool(name="weights", bufs=1))
            apool = ctx.enter_context(tc.tile_pool(name="acts", bufs=1))
            spool = ctx.enter_context(tc.tile_pool(name="scratch", bufs=1))
            pspool = ctx.enter_context(tc.tile_pool(name="ps", bufs=8, space="PSUM"))

            ones16 = cpool.tile([128, 128], DT_MM, tag="ones16", name="ones16")
            nc.vector.memset(ones16[:], 1.0)
            eps_t = cpool.tile([1, 1], f32, tag="eps", name="eps_t")
            nc.vector.memset(eps_t[:], EPS)
            maskc = cpool.tile([128, BS * KT], f32, tag="maskc", name="maskc")
            nc.sync.dma_start(maskc[:], maskc_d[:])

            # initial h: per (k, ci) fp16 [128, 400] tiles (dim-major)
            h16 = {}
            for k in range(KT):
                for ci, (co, cs) in enumerate(MMCH):
                    t = apool.tile([128, 400], DT_MM, tag="h16", bufs=2 * KT,
                                   name=f"h16_0_{k}_{ci}")
                    nc.sync.dma_start(t[:], h0_d[k, :, co:co + cs])
                    h16[(k, ci)] = t

            def layer_norm(x16, ci, ln_i, bcol_t, grow_t, l, tag, last=False):
                """Dim-major LN of one token-chunk (KT fp16 [128,400] tiles)."""
                sq = []
                for k in range(KT):
                    s = spool.tile([128, 400], DT_MM, tag="sq", bufs=2 * KT,
                                   name=f"{tag}sq{l}_{ci}_{k}")
                    nc.scalar.activation(s[:], x16[k][:], AF.Square)
                    sq.append(s)
                outs = [apool.tile([128, 400], DT_MM, tag=f"{tag}o", bufs=4 * KT,
                                   name=f"{tag}o{l}_{ci}_{k}") for k in range(KT)]
                ssum = pspool.tile([1, 400], f32, tag="ps", name=f"{tag}ss{l}_{ci}")
                for k in range(KT):
                    nc.tensor.matmul(ssum[:], ones16[:, 0:1], x16[k][:],
                                     start=(k == 0), stop=(k == KT - 1))
                ssq = pspool.tile([1, 400], f32, tag="ps", name=f"{tag}sk{l}_{ci}")
                for k in range(KT):
                    nc.tensor.matmul(ssq[:], ones16[:, 0:1], sq[k][:],
                                     start=(k == 0), stop=(k == KT - 1))
                nm = spool.tile([1, 400], f32, tag="lnrow", bufs=6,
                                name=f"{tag}nm{l}_{ci}")
                nc.vector.tensor_scalar_mul(nm[:], ssum[:], -1.0 / D)
                msq = spool.tile([1, 400], f32, tag="lnrow", bufs=6,
                                 name=f"{tag}ms{l}_{ci}")
                nc.scalar.activation(msq[:], ssum[:], AF.Square, scale=-1.0 / D)
                var = spool.tile([1, 400], f32, tag="lnrow", bufs=6,
                                 name=f"{tag}va{l}_{ci}")
                nc.vector.scalar_tensor_tensor(
                    var[:], ssq[:], 1.0 / D, msq[:], op0=ALU.mult,
                    op1=ALU.subtract)
                lnv = spool.tile([1, 400], f32, tag="lnrow", bufs=6,
                                 name=f"{tag}lv{l}_{ci}")
                nc.scalar.activation(lnv[:], var[:], AF.Ln, bias=eps_t[:])
                rstd = spool.tile([1, 400], DT_MM, tag="lnrow16", bufs=4,
                                  name=f"{tag}rs{l}_{ci}")
                nc.scalar.activation(rstd[:], lnv[:], AF.Exp, scale=-0.5)
                mr = spool.tile([1, 400], DT_MM, tag="lnrow16", bufs=4,
                                name=f"{tag}mr{l}_{ci}")
                nc.vector.tensor_tensor(mr[:], nm[:], rstd[:], op=ALU.mult)
                for k in range(KT):
                    gsl = grow_t[0:1, (ln_i * KT + k) * 128:(ln_i * KT + k + 1) * 128]
                    rbg = pspool.tile([128, 400], f32, tag="ps",
                                      name=f"{tag}rb{l}_{ci}_{k}")
                    nc.tensor.matmul(rbg[:], gsl, rstd[:], start=True, stop=True)
                    mbg = pspool.tile([128, 400], f32, tag="ps",
                                      name=f"{tag}mb{l}_{ci}_{k}")
                    nc.tensor.matmul(mbg[:], gsl, mr[:], start=True, stop=True)
                    u = spool.tile([128, 400], f32, tag="uv", bufs=3,
                                   name=f"{tag}u{l}_{ci}_{k}")
                    nc.vector.tensor_tensor(u[:], x16[k][:], rbg[:], op=ALU.mult)
                    bsl = bcol_t[:, ln_i * KT + k:ln_i * KT + k + 1]
                    nc.vector.scalar_tensor_tensor(
                        outs[k][:], u[:], bsl, mbg[:], op0=ALU.add, op1=ALU.add)
                    if last:
                        outf = spool.tile([128, 400], f32, tag="outf", bufs=KT,
                                          name=f"outf_{ci}_{k}")
                        nc.vector.scalar_tensor_tensor(
                            outf[:], u[:], bsl, mbg[:], op0=ALU.add, op1=ALU.add)
                        co = MMCH[ci][0]
                        nc.sync.dma_start(out_d[k, :, co:co + 400], outf[:])
                return outs

            for l in range(L_RUN):
                # ---- layer weights ----
                wqk_t = wpool.tile([128, KT, 2 * H * D], DT_MM, tag="wqk", bufs=1,
                                   name=f"wqk{l}")
                nc.sync.dma_start(wqk_t[:], wqk_d[l])
                wv_t = wpool.tile([128, KT, H * D], DT_MM, tag="wv", bufs=2,
                                  name=f"wv{l}")
                nc.sync.dma_start(wv_t[:], wv_d[l])
                ow_t = wpool.tile([128, NCTX, D], DT_MM, tag="ow", bufs=2,
                                  name=f"ow{l}")
                nc.sync.dma_start(ow_t[:], ow_d[l])
                ff1_t = wpool.tile([128, KT, DFF], DT_MM, tag="ff1", bufs=2,
                                   name=f"ff1{l}")
                nc.sync.dma_start(ff1_t[:], ff1_d[l])
                ff2_t = wpool.tile([128, NMID, D], DT_MM, tag="ff2", bufs=2,
                                   name=f"ff2{l}")
                nc.sync.dma_start(ff2_t[:], ff2_d[l])
                bqk_t = wpool.tile([128, NQK], f32, tag="bqk", bufs=2, name=f"bqk{l}")
                nc.sync.dma_start(bqk_t[:], bqk_d[l])
                obe_t = wpool.tile([128, KT], f32, tag="obe", bufs=2, name=f"obe{l}")
                nc.sync.dma_start(obe_t[:], obe_d[l])
                f1b_t = wpool.tile([128, NMID], f32, tag="f1b", bufs=2, name=f"f1b{l}")
                nc.sync.dma_start(f1b_t[:], f1b_d[l])
                f2b_t = wpool.tile([128, KT], f32, tag="f2b", bufs=2, name=f"f2b{l}")
                nc.sync.dma_start(f2b_t[:], f2b_d[l])
                bcol_t = wpool.tile([128, 2 * KT], f32, tag="bcol", bufs=2,
                                    name=f"bcol{l}")
                nc.sync.dma_start(bcol_t[:], bcol_d[l])
                grow_t = wpool.tile([1, 2 * KT * 128], DT_MM, tag="grow", bufs=2,
                                    name=f"grow{l}")
                nc.sync.dma_start(grow_t[:], grow_d[l])

                # ---- QK: dim-major fp16, per (m, ci) [128,400] tiles ----
                qk = {}
                for ci in range(2):
                    for m in range(NQK):
                        qt = apool.tile([128, 400], DT_MM, tag="qk", bufs=2 * NQK,
                                        name=f"qk{l}_{m}_{ci}")
                        ps = pspool.tile([128, 400], f32, tag="ps",
                                         name=f"psqk{l}_{m}_{ci}")
                        for k in range(KT):
                            nc.tensor.matmul(
                                ps[:], wqk_t[:, k, m * 128:(m + 1) * 128],
                                h16[(k, ci)][:],
                                start=(k == 0), stop=(k == KT - 1))
                        if m % 2 == 0:
                            nc.scalar.activation(qt[:], ps[:], AF.Identity,
                                                 bias=bqk_t[:, m:m + 1])
                        else:
                            nc.vector.tensor_scalar_add(qt[:], ps[:],
                                                        bqk_t[:, m:m + 1])
                        qk[(m, ci)] = qt

                ctx_big = {ci: apool.tile([128, NCTX, 400], DT_MM, tag="ctxb",
                                          bufs=2, name=f"ctxb{l}_{ci}")
                           for ci in range(2)}

                def att_seq(b):
                    """V + attention for one sequence (transpose-free)."""
                    ci, sb = b // 2, (b % 2) * S
                    vt = {}
                    for ti, (to, ts) in enumerate(SEQ_TILES):
                        v = apool.tile([128, H * D], DT_MM, tag="v", bufs=2 * BS,
                                       name=f"v{l}_{b}_{ti}")
                        for nch in range(3):
                            ps = pspool.tile([128, 512], f32, tag="ps",
                                             name=f"psv{l}_{b}_{ti}_{nch}")
                            for k in range(KT):
                                nc.tensor.matmul(
                                    ps[0:ts, :],
                                    h16[(k, ci)][:, sb + to:sb + to + ts],
                                    wv_t[:, k, nch * 512:(nch + 1) * 512],
                                    start=(k == 0), stop=(k == KT - 1))
                            nc.scalar.activation(
                                v[0:ts, nch * 512:(nch + 1) * 512], ps[0:ts, :],
                                AF.Copy)
                        vt[ti] = v
                    for h in range(H):
                        sps = pspool.tile([128, 400], f32, tag="ps",
                                          name=f"sps{l}_{b}_{h}")
                        for ti, (to, ts) in enumerate(SEQ_TILES):
                            for k in range(KT):
                                nc.tensor.matmul(
                                    sps[0:ts, ti * S:(ti + 1) * S],
                                    qk[((H + h) * KT + k, ci)][:, sb + to:sb + to + ts],
                                    qk[(h * KT + k, ci)][:, sb:sb + S],
                                    start=(k == 0), stop=(k == KT - 1))
                        at = spool.tile([128, 400], DT_MM, tag="attn", bufs=6,
                                        name=f"at{l}_{b}_{h}")
                        for ti, (to, ts) in enumerate(SEQ_TILES):
                            nc.scalar.activation(
                                at[0:ts, ti * S:(ti + 1) * S],
                                sps[0:ts, ti * S:(ti + 1) * S], AF.Exp,
                                bias=maskc[0:ts, b * KT + ti:b * KT + ti + 1])
                        dps = pspool.tile([128, S], f32, tag="ps",
                                          name=f"dps{l}_{b}_{h}")
                        for ti, (to, ts) in enumerate(SEQ_TILES):
                            nc.tensor.matmul(dps[:], ones16[0:ts, :],
                                             at[0:ts, ti * S:(ti + 1) * S],
                                             start=(ti == 0), stop=(ti == 1))
                        rden = spool.tile([128, S], f32, tag="rden", bufs=3,
                                          name=f"rden{l}_{b}_{h}")
                        nc.vector.reciprocal_approx_fast(rden[:], dps[:])
                        cps = pspool.tile([128, 400], f32, tag="ps",
                                          name=f"cps{l}_{b}_{h}")
                        for dvh in range(2):
                            for ti, (to, ts) in enumerate(SEQ_TILES):
                                nc.tensor.matmul(
                                    cps[:, dvh * S:(dvh + 1) * S],
                                    vt[ti][0:ts,
                                           h * D + dvh * 128:h * D + (dvh + 1) * 128],
                                    at[0:ts, ti * S:(ti + 1) * S],
                                    start=(ti == 0), stop=(ti == 1))
                        nc.vector.tensor_tensor(
                            ctx_big[ci][:, 2 * h:2 * h + 2, sb:sb + S],
                            cps[:].rearrange("p (v s) -> p v s", v=2),
                            rden[:].rearrange("p (o s) -> p o s", o=1)
                                   .broadcast_to([128, 2, S]),
                            op=ALU.mult)

                def tail1(ci):
                    """out-projection + residual + LN1 for one chunk."""
                    x16 = []
                    for d2 in range(KT):
                        xt = apool.tile([128, 400], DT_MM, tag="x16", bufs=2 * KT,
                                        name=f"x16_{l}_{d2}_{ci}")
                        ps = pspool.tile([128, 400], f32, tag="ps",
                                         name=f"pso{l}_{d2}_{ci}")
                        for kt in range(NCTX):
                            nc.tensor.matmul(
                                ps[:], ow_t[:, kt, d2 * 128:(d2 + 1) * 128],
                                ctx_big[ci][:, kt, :],
                                start=(kt == 0), stop=(kt == NCTX - 1))
                        nc.vector.scalar_tensor_tensor(
                            xt[:], ps[:], obe_t[:, d2:d2 + 1], h16[(d2, ci)][:],
                            op0=ALU.add, op1=ALU.add)
                        x16.append(xt)
                    return layer_norm(x16, ci, 0, bcol_t, grow_t, l, "ln1")

                def tail2(ci, o1c, last):
                    """FFN + residual + LN2 for one chunk."""
                    mid = []
                    for m in range(NMID):
                        mt = apool.tile([128, 400], DT_MM, tag="mid", bufs=2 * NMID,
                                        name=f"mid{l}_{m}_{ci}")
                        ps = pspool.tile([128, 400], f32, tag="ps",
                                         name=f"psf1{l}_{m}_{ci}")
                        for k in range(KT):
                            nc.tensor.matmul(
                                ps[:], ff1_t[:, k, m * 128:(m + 1) * 128],
                                o1c[k][:],
                                start=(k == 0), stop=(k == KT - 1))
                        if m % 2 == 0:
                            nc.scalar.activation(mt[:], ps[:], AF.Relu,
                                                 bias=f1b_t[:, m:m + 1])
                        else:
                            nc.vector.tensor_scalar(
                                mt[:], ps[:], f1b_t[:, m:m + 1], 0.0,
                                op0=ALU.add, op1=ALU.max)
                        mid.append(mt)
                    x2c = []
                    for d2 in range(KT):
                        xt = apool.tile([128, 400], DT_MM, tag="x2", bufs=2 * KT,
                                        name=f"x2_{l}_{d2}_{ci}")
                        ps = pspool.tile([128, 400], f32, tag="ps",
                                         name=f"psf2{l}_{d2}_{ci}")
                        for kt in range(NMID):
                            nc.tensor.matmul(
                                ps[:], ff2_t[:, kt, d2 * 128:(d2 + 1) * 128],
                                mid[kt][:],
                                start=(kt == 0), stop=(kt == NMID - 1))
                        nc.vector.scalar_tensor_tensor(
                            xt[:], ps[:], f2b_t[:, d2:d2 + 1], o1c[d2][:],
                            op0=ALU.add, op1=ALU.add)
                        x2c.append(xt)
                    return layer_norm(x2c, ci, 1, bcol_t, grow_t, l, "ln2",
                                      last=last)

                # staggered emission: attention blocks fill LN-chain gaps
                last = (l == L_RUN - 1)
                att_seq(0)
                att_seq(1)
                o1_0 = tail1(0)
                att_seq(2)
                h_0 = tail2(0, o1_0, last)
                att_seq(3)
                o1_1 = tail1(1)
                h_1 = tail2(1, o1_1, last)
                newh = {}
                for k in range(KT):
                    newh[(k, 0)] = h_0[k]
                    newh[(k, 1)] = h_1[k]
                h16 = newh

    nc.compile()
    return nc


def _fold_weights(wqkv_w, wqkv_b, A1, A2, A3, A4, tnb, out_w, out_b):
    """Fold the TN contraction into dense weights; fold v-bias into out bias;
    fold 1/sqrt(D) into Q."""
    wqkv_w = np.asarray(wqkv_w, np.float32)
    wqkv_b = np.asarray(wqkv_b, np.float32)
    out_w = np.asarray(out_w, np.float32)
    out_b = np.asarray(out_b, np.float32)
    tnb = np.asarray(tnb, np.float32)
    scale = 1.0 / np.sqrt(np.float32(D))

    W_full = np.zeros((L, 3, D, H * D), np.float32)
    b_full = np.zeros((L, 3, H * D), np.float32)
    for l in range(L):
        for x in range(3):
            wt = np.einsum('pmi,qmnj,rnok,tol->pqrtijkl',
                           np.asarray(A1[l, x], np.float64),
                           np.asarray(A2[l, x], np.float64),
                           np.asarray(A3[l, x], np.float64),
                           np.asarray(A4[l, x], np.float64),
                           optimize=True).reshape(D, 4 * D).astype(np.float32)
            W_full[l, x] = np.concatenate([wqkv_w[l, x], wt], axis=1)
            b_full[l, x] = np.concatenate([wqkv_b[l, x], tnb[l, x]])
    W_full[:, 0] *= scale
    b_full[:, 0] *= scale

    wqk = np.concatenate([W_full[:, 0], W_full[:, 1]], axis=2)   # [L, 256, 3072]
    bqk = np.concatenate([b_full[:, 0], b_full[:, 1]], axis=1)   # [L, 3072]
    wv = W_full[:, 2]                                            # [L, 256, 1536]
    bv = b_full[:, 2]                                            # [L, 1536]
    obe = out_b + np.einsum('lc,lcd->ld', bv, out_w)             # [L, 256]
    return wqk, bqk, wv, obe


def _pack_w(x, nk):
    """[L, nk*128, M] -> [L, 128, nk, M] (partition-major SBUF layout)."""
    Lh, K, M = x.shape
    return np.ascontiguousarray(
        x.reshape(Lh, nk, 128, M).transpose(0, 2, 1, 3))


def _pack_cols(x, n):
    """[L, n*128] -> [L, 128, n]."""
    return np.ascontiguousarray(x.reshape(L, n, 128).transpose(0, 2, 1))


def kernel(**inputs):
    tokens = np.asarray(inputs["tokens"])
    tok_emb = np.asarray(inputs["tok_emb"], np.float32)
    pos_emb = np.asarray(inputs["pos_emb"], np.float32)

    wqk, bqk, wv, obe = _fold_weights(
        inputs["wqkv_w"], inputs["wqkv_b"], inputs["A1"], inputs["A2"],
        inputs["A3"], inputs["A4"], inputs["tnb"], inputs["out_w"],
        inputs["out_b"])
    ff1 = np.asarray(inputs["ff1_w"], np.float32)
    f1b = np.asarray(inputs["ff1_b"], np.float32)
    ff2 = np.asarray(inputs["ff2_w"], np.float32)
    f2b = np.asarray(inputs["ff2_b"], np.float32)
    ow = np.asarray(inputs["out_w"], np.float32)

    # LN biases as per-partition cols [L,128,2KT]; gains as fp16 rows
    bcol = np.stack([np.asarray(inputs["ln1_b"], np.float32),
                     np.asarray(inputs["ln2_b"], np.float32)], axis=1)  # [L,2,256]
    bcol = np.ascontiguousarray(
        bcol.reshape(L, 2, KT, 128).transpose(0, 3, 1, 2).reshape(L, 128, 2 * KT))
    grow = np.stack([np.asarray(inputs["ln1_g"], np.float32),
                     np.asarray(inputs["ln2_g"], np.float32)], axis=1)  # [L,2,256]
    grow = np.ascontiguousarray(grow.reshape(L, 1, 2 * KT * 128)).astype(NP_MM)

    shared = {
        "wqk": _pack_w(wqk.astype(NP_MM), KT),
        "bqk": _pack_cols(bqk, NQK),
        "wv": _pack_w(wv.astype(NP_MM), KT),
        "obe": _pack_cols(obe, KT),
        "ow": _pack_w(ow.astype(NP_MM), NCTX),
        "ff1": _pack_w(ff1.astype(NP_MM), KT),
        "f1b": _pack_cols(f1b, NMID),
        "ff2": _pack_w(ff2.astype(NP_MM), NMID),
        "f2b": _pack_cols(f2b, KT),
        "bcol": bcol,
        "grow": grow,
    }

    h0 = tok_emb[tokens] + pos_emb[None]          # [B, S, D] f32
    maskbias = np.where(tokens == 0, np.float32(-1e9),
                        np.float32(0.0)) - np.float32(CSHIFT)   # [B,S]

    in_maps = []
    for c in range(N_CORES):
        hc = h0[c * BS:(c + 1) * BS].reshape(T, D)
        hdim = np.ascontiguousarray(hc.T.reshape(KT, 128, T)).astype(NP_MM)
        mc = np.full((128, BS * KT), -1e9, np.float32)
        for b in range(BS):
            mb = maskbias[c * BS + b]             # [S]
            mc[0:128, b * KT + 0] = mb[0:128]
            mc[0:72, b * KT + 1] = mb[128:200]
        m = dict(shared)
        m["h0"] = hdim
        m["maskc"] = np.ascontiguousarray(mc)
        in_maps.append(m)

    if "nc" not in _CACHE:
        _CACHE["nc"] = _build_program()
    nc = _CACHE["nc"]
    _CACHE["in_maps"] = in_maps

    res = run_bass_kernel_spmd(nc, in_maps, list(range(N_CORES)))
    outs = []
    for c in range(N_CORES):
        od = res.results[c]["out"].reshape(D, T)      # dim-major
        outs.append(od.T.reshape(BS, S, D))
    return np.concatenate(outs, axis=0).astype(np.float32)


if __name__ == "__main__":
    import reference
    inputs = {k: np.asarray(v) for k, v in reference.setup_inputs().items()}
    got = kernel(**inputs)
    exp = np.asarray(reference.reference(**inputs))
    err = np.abs(got - exp).max() / np.abs(exp).max()
    print(f"Relative error: {err:.3e}")


# revision 16
# speedup vs baseline: 1.0926x; 1.0926x over previous
"""Trainium2 Bass kernel for nn_BERT_tensor (8-layer BERT with tensor-network heads).

Strategy (v3):
  - Data-parallel over batch: 32 seqs -> 4 seqs (800 tokens) per core x 8 cores.
  - Host folds the MPO tensor-network contraction (A1..A4) into a dense
    [256 -> 1024] weight per (layer, q/k/v); QKV is one dense matmul.
  - Everything stays DIM-MAJOR ([dim, token]); zero PE transposes:
    * attention computes scores TRANSPOSED ([kpos, qpos]); softmax
      max-subtraction replaced by a constant shift (scores bounded ~|14|);
      denominator via ones-matmul; division via reciprocal_approx_fast;
      normalization fused into ctx evacuation.
    * LayerNorm dim-major: stats via ones-matmuls over the partition axis;
      rstd = exp(-0.5*ln(var+eps)) on ScalarE (no Sqrt table set, no slow
      DVE reciprocal); per-token scale/offset broadcast via rank-1
      outer-product matmuls with the LN gain as the stationary operand.
  - fp16 matmul inputs everywhere (fp32 PSUM accumulation).
  - Single 1-bank PSUM pool (8 slots), 400-col chunks for fine-grained
    pipelining to keep TensorE dense (HAM warm).
"""
import numpy as np
from contextlib import ExitStack

import concourse.bass as bass
import concourse.bacc as bacc
import concourse.tile as tile
import concourse.mybir as mybir
from concourse.bass_utils import run_bass_kernel_spmd

dt = mybir.dt
AF = mybir.ActivationFunctionType
ALU = mybir.AluOpType

# problem constants (hardcoded per contract)
B, S, D = 32, 200, 256
H, DFF, VOCAB, L, TD = 6, 1024, 3500, 8, 2
N_CORES = 8
BS = B // N_CORES            # 4 seqs per core
T = BS * S                   # 800 tokens per core
KT = D // 128                # 2 k-tiles over emb dim
NQK = (2 * H * D) // 128     # 24 m-tiles over Q|K outdim (3072)
NCTX = (H * D) // 128        # 12 tiles over ctx dim (1536)
NMID = DFF // 128            # 8 tiles over ffn hidden
MMCH = [(0, 400), (400, 400)]       # N chunks of T (each fits one PSUM bank)
SEQ_TILES = [(0, 128), (128, 72)]   # per-seq kpos tiles
EPS = 1e-6
CSHIFT = 6.0                 # softmax constant shift (max |score| ~ 14)

import os
L_RUN = int(os.environ.get("BERT_L_RUN", str(L)))
DT_MM = dt.float16
NP_MM = np.float16

_CACHE = {}


def _patch_act_tables():
    """Force every activation function to resolve to the
    natural_log_exp_and_others table set (it contains exp/ln/square/
    identity/copy/relu — everything this kernel uses), so exactly one
    ACT_TABLE_LOAD is emitted instead of one per exp<->ln alternation.
    Set names and their act_info.json indices are preserved."""
    import concourse.hw_specs as hw_specs
    import concourse.bacc as bacc_mod
    if getattr(bacc_mod, "_act_tables_patched", False):
        return
    orig = hw_specs.get_activation_tables

    def only_nle(arch):
        t = orig(arch)
        return {k: (v if k == "natural_log_exp_and_others" else set())
                for k, v in t.items()}

    bacc_mod.get_activation_tables = only_nle
    bacc_mod._act_tables_patched = True


def _build_program():
    _patch_act_tables()
    nc = bacc.Bacc("TRN2", target_bir_lowering=False, debug=False,
                   num_devices=N_CORES)

    f32 = dt.float32
    inp = {}

    def din(name, shape, dty):
        inp[name] = nc.dram_tensor(name, list(shape), dty, kind="ExternalInput").ap()
        return inp[name]

    h0_d = din("h0", [KT, 128, T], DT_MM)
    maskc_d = din("maskc", [128, BS * KT], f32)        # exp bias cols (mask - C)
    wqk_d = din("wqk", [L, 128, KT, 2 * H * D], DT_MM)
    bqk_d = din("bqk", [L, 128, NQK], f32)
    wv_d = din("wv", [L, 128, KT, H * D], DT_MM)
    ow_d = din("ow", [L, 128, NCTX, D], DT_MM)
    obe_d = din("obe", [L, 128, KT], f32)
    ff1_d = din("ff1", [L, 128, KT, DFF], DT_MM)
    f1b_d = din("f1b", [L, 128, NMID], f32)
    ff2_d = din("ff2", [L, 128, NMID, D], DT_MM)
    f2b_d = din("f2b", [L, 128, KT], f32)
    bcol_d = din("bcol", [L, 128, 2 * KT], f32)        # b1|b2 cols
    grow_d = din("grow", [L, 1, 2 * KT * 128], DT_MM)  # g1|g2 rows (fp16)
    out_d = nc.dram_tensor("out", [KT, 128, T], f32, kind="ExternalOutput").ap()

    with tile.TileContext(nc) as tc:
        with ExitStack() as ctx:
            cpool = ctx.enter_context(tc.tile_pool(name="const", bufs=1))
            wpool = ctx.enter_context(tc.tile_pool(name="weights", bufs=1))
            apool = ctx.enter_context(tc.tile_pool(name="acts", bufs=1))
            spool = ctx.enter_context(tc.tile_pool(name="scratch", bufs=1))
            psA = ctx.enter_context(tc.tile_pool(name="psA", bufs=4, space="PSUM"))
            psB = ctx.enter_context(tc.tile_pool(name="psB", bufs=2, space="PSUM"))
            psC = ctx.enter_context(tc.tile_pool(name="psC", bufs=2, space="PSUM"))

            ones16 = cpool.tile([128, 128], DT_MM, tag="ones16", name="ones16")
            nc.vector.memset(ones16[:], 1.0)
            eps_t = cpool.tile([1, 1], f32, tag="eps", name="eps_t")
            nc.vector.memset(eps_t[:], EPS)
            maskc = cpool.tile([128, BS * KT], f32, tag="maskc", name="maskc")
            nc.sync.dma_start(maskc[:], maskc_d[:])

            # initial h: per (k, ci) fp16 [128, 400] tiles (dim-major)
            h16 = {}
            for k in range(KT):
                for ci, (co, cs) in enumerate(MMCH):
                    t = apool.tile([128, 400], DT_MM, tag="h16", bufs=2 * KT,
                                   name=f"h16_0_{k}_{ci}")
                    nc.sync.dma_start(t[:], h0_d[k, :, co:co + cs])
                    h16[(k, ci)] = t

            def layer_norm(x16, ci, ln_i, bcol_t, grow_t, l, tag, last=False):
                """Dim-major LN of one token-chunk (KT fp16 [128,400] tiles)."""
                sq = []
                for k in range(KT):
                    s = spool.tile([128, 400], DT_MM, tag="sq", bufs=2 * KT,
                                   name=f"{tag}sq{l}_{ci}_{k}")
                    nc.scalar.activation(s[:], x16[k][:], AF.Square)
                    sq.append(s)
                outs = [apool.tile([128, 400], DT_MM, tag=f"{tag}o", bufs=4 * KT,
                                   name=f"{tag}o{l}_{ci}_{k}") for k in range(KT)]
                ssum = psC.tile([1, 400], f32, tag="ps", name=f"{tag}ss{l}_{ci}")
                for k in range(KT):
                    nc.tensor.matmul(ssum[:], ones16[:, 0:1], x16[k][:],
                                     start=(k == 0), stop=(k == KT - 1))
                ssq = psC.tile([1, 400], f32, tag="ps", name=f"{tag}sk{l}_{ci}")
                for k in range(KT):
                    nc.tensor.matmul(ssq[:], ones16[:, 0:1], sq[k][:],
                                     start=(k == 0), stop=(k == KT - 1))
                nm = spool.tile([1, 400], f32, tag="lnrow", bufs=6,
                                name=f"{tag}nm{l}_{ci}")
                nc.vector.tensor_scalar_mul(nm[:], ssum[:], -1.0 / D)
                msq = spool.tile([1, 400], f32, tag="lnrow", bufs=6,
                                 name=f"{tag}ms{l}_{ci}")
                nc.scalar.activation(msq[:], ssum[:], AF.Square, scale=-1.0 / D)
                var = spool.tile([1, 400], f32, tag="lnrow", bufs=6,
                                 name=f"{tag}va{l}_{ci}")
                nc.vector.scalar_tensor_tensor(
                    var[:], ssq[:], 1.0 / D, msq[:], op0=ALU.mult,
                    op1=ALU.subtract)
                lnv = spool.tile([1, 400], f32, tag="lnrow", bufs=6,
                                 name=f"{tag}lv{l}_{ci}")
                nc.scalar.activation(lnv[:], var[:], AF.Ln, bias=eps_t[:])
                rstd = spool.tile([1, 400], DT_MM, tag="lnrow16", bufs=4,
                                  name=f"{tag}rs{l}_{ci}")
                nc.scalar.activation(rstd[:], lnv[:], AF.Exp, scale=-0.5)
                mr = spool.tile([1, 400], DT_MM, tag="lnrow16", bufs=4,
                                name=f"{tag}mr{l}_{ci}")
                nc.vector.tensor_tensor(mr[:], nm[:], rstd[:], op=ALU.mult)
                for k in range(KT):
                    gsl = grow_t[0:1, (ln_i * KT + k) * 128:(ln_i * KT + k + 1) * 128]
                    rbg = psC.tile([128, 400], f32, tag="ps",
                                      name=f"{tag}rb{l}_{ci}_{k}")
                    nc.tensor.matmul(rbg[:], gsl, rstd[:], start=True, stop=True)
                    mbg = psC.tile([128, 400], f32, tag="ps",
                                      name=f"{tag}mb{l}_{ci}_{k}")
                    nc.tensor.matmul(mbg[:], gsl, mr[:], start=True, stop=True)
                    u = spool.tile([128, 400], f32, tag="uv", bufs=3,
                                   name=f"{tag}u{l}_{ci}_{k}")
                    nc.vector.tensor_tensor(u[:], x16[k][:], rbg[:], op=ALU.mult)
                    bsl = bcol_t[:, ln_i * KT + k:ln_i * KT + k + 1]
                    nc.vector.scalar_tensor_tensor(
                        outs[k][:], u[:], bsl, mbg[:], op0=ALU.add, op1=ALU.add)
                    if last:
                        outf = spool.tile([128, 400], f32, tag="outf", bufs=KT,
                                          name=f"outf_{ci}_{k}")
                        nc.vector.scalar_tensor_tensor(
                            outf[:], u[:], bsl, mbg[:], op0=ALU.add, op1=ALU.add)
                        co = MMCH[ci][0]
                        nc.sync.dma_start(out_d[k, :, co:co + 400], outf[:])
                return outs

            for l in range(L_RUN):
                # ---- layer weights ----
                wqk_t = wpool.tile([128, KT, 2 * H * D], DT_MM, tag="wqk", bufs=1,
                                   name=f"wqk{l}")
                nc.sync.dma_start(wqk_t[:], wqk_d[l])
                wv_t = wpool.tile([128, KT, H * D], DT_MM, tag="wv", bufs=2,
                                  name=f"wv{l}")
                nc.sync.dma_start(wv_t[:], wv_d[l])
                ow_t = wpool.tile([128, NCTX, D], DT_MM, tag="ow", bufs=2,
                                  name=f"ow{l}")
                nc.sync.dma_start(ow_t[:], ow_d[l])
                ff1_t = wpool.tile([128, KT, DFF], DT_MM, tag="ff1", bufs=2,
                                   name=f"ff1{l}")
                nc.sync.dma_start(ff1_t[:], ff1_d[l])
                ff2_t = wpool.tile([128, NMID, D], DT_MM, tag="ff2", bufs=2,
                                   name=f"ff2{l}")
                nc.sync.dma_start(ff2_t[:], ff2_d[l])
                bqk_t = wpool.tile([128, NQK], f32, tag="bqk", bufs=2, name=f"bqk{l}")
                nc.sync.dma_start(bqk_t[:], bqk_d[l])
                obe_t = wpool.tile([128, KT], f32, tag="obe", bufs=2, name=f"obe{l}")
                nc.sync.dma_start(obe_t[:], obe_d[l])
                f1b_t = wpool.tile([128, NMID], f32, tag="f1b", bufs=2, name=f"f1b{l}")
                nc.sync.dma_start(f1b_t[:], f1b_d[l])
                f2b_t = wpool.tile([128, KT], f32, tag="f2b", bufs=2, name=f"f2b{l}")
                nc.sync.dma_start(f2b_t[:], f2b_d[l])
                bcol_t = wpool.tile([128, 2 * KT], f32, tag="bcol", bufs=2,
                                    name=f"bcol{l}")
                nc.sync.dma_start(bcol_t[:], bcol_d[l])
                grow_t = wpool.tile([1, 2 * KT * 128], DT_MM, tag="grow", bufs=2,
                                    name=f"grow{l}")
                nc.sync.dma_start(grow_t[:], grow_d[l])

                # ---- QK: dim-major fp16, per (m, ci) [128,400] tiles ----
                qk = {}
                for ci in range(2):
                    for m in range(NQK):
                        qt = apool.tile([128, 400], DT_MM, tag="qk", bufs=2 * NQK,
                                        name=f"qk{l}_{m}_{ci}")
                        ps = psB.tile([128, 400], f32, tag="ps",
                                         name=f"psqk{l}_{m}_{ci}")
                        for k in range(KT):
                            nc.tensor.matmul(
                                ps[:], wqk_t[:, k, m * 128:(m + 1) * 128],
                                h16[(k, ci)][:],
                                start=(k == 0), stop=(k == KT - 1))
                        if m % 2 == 0:
                            nc.scalar.activation(qt[:], ps[:], AF.Identity,
                                                 bias=bqk_t[:, m:m + 1])
                        else:
                            nc.vector.tensor_scalar_add(qt[:], ps[:],
                                                        bqk_t[:, m:m + 1])
                        qk[(m, ci)] = qt

                ctx_big = {ci: apool.tile([128, NCTX, 400], DT_MM, tag="ctxb",
                                          bufs=2, name=f"ctxb{l}_{ci}")
                           for ci in range(2)}

                def att_seq(b):
                    """V + attention for one sequence (transpose-free)."""
                    ci, sb = b // 2, (b % 2) * S
                    vt = {}
                    for ti, (to, ts) in enumerate(SEQ_TILES):
                        v = apool.tile([128, H * D], DT_MM, tag="v", bufs=2 * BS,
                                       name=f"v{l}_{b}_{ti}")
                        for nch in range(3):
                            ps = psA.tile([128, 512], f32, tag="ps",
                                             name=f"psv{l}_{b}_{ti}_{nch}")
                            for k in range(KT):
                                nc.tensor.matmul(
                                    ps[0:ts, :],
                                    h16[(k, ci)][:, sb + to:sb + to + ts],
                                    wv_t[:, k, nch * 512:(nch + 1) * 512],
                                    start=(k == 0), stop=(k == KT - 1))
                            nc.scalar.activation(
                                v[0:ts, nch * 512:(nch + 1) * 512], ps[0:ts, :],
                                AF.Copy)
                        vt[ti] = v
                    for h in range(H):
                        sps = psA.tile([128, 400], f32, tag="ps",
                                          name=f"sps{l}_{b}_{h}")
                        for ti, (to, ts) in enumerate(SEQ_TILES):
                            for k in range(KT):
                                nc.tensor.matmul(
                                    sps[0:ts, ti * S:(ti + 1) * S],
                                    qk[((H + h) * KT + k, ci)][:, sb + to:sb + to + ts],
                                    qk[(h * KT + k, ci)][:, sb:sb + S],
                                    start=(k == 0), stop=(k == KT - 1))
                        at = spool.tile([128, 400], DT_MM, tag="attn", bufs=6,
                                        name=f"at{l}_{b}_{h}")
                        for ti, (to, ts) in enumerate(SEQ_TILES):
                            nc.scalar.activation(
                                at[0:ts, ti * S:(ti + 1) * S],
                                sps[0:ts, ti * S:(ti + 1) * S], AF.Exp,
                                bias=maskc[0:ts, b * KT + ti:b * KT + ti + 1])
                        dps = psA.tile([128, S], f32, tag="ps",
                                          name=f"dps{l}_{b}_{h}")
                        for ti, (to, ts) in enumerate(SEQ_TILES):
                            nc.tensor.matmul(dps[:], ones16[0:ts, :],
                                             at[0:ts, ti * S:(ti + 1) * S],
                                             start=(ti == 0), stop=(ti == 1))
                        rden = spool.tile([128, S], f32, tag="rden", bufs=3,
                                          name=f"rden{l}_{b}_{h}")
                        nc.vector.reciprocal_approx_fast(rden[:], dps[:])
                        cps = psA.tile([128, 400], f32, tag="ps",
                                          name=f"cps{l}_{b}_{h}")
                        for dvh in range(2):
                            for ti, (to, ts) in enumerate(SEQ_TILES):
                                nc.tensor.matmul(
                                    cps[:, dvh * S:(dvh + 1) * S],
                                    vt[ti][0:ts,
                                           h * D + dvh * 128:h * D + (dvh + 1) * 128],
                                    at[0:ts, ti * S:(ti + 1) * S],
                                    start=(ti == 0), stop=(ti == 1))
                        nc.vector.tensor_tensor(
                            ctx_big[ci][:, 2 * h:2 * h + 2, sb:sb + S],
                            cps[:].rearrange("p (v s) -> p v s", v=2),
                            rden[:].rearrange("p (o s) -> p o s", o=1)
                                   .broadcast_to([128, 2, S]),
                            op=ALU.mult)

                def tail1(ci):
                    """out-projection + residual + LN1 for one chunk."""
                    x16 = []
                    for d2 in range(KT):
                        xt = apool.tile([128, 400], DT_MM, tag="x16", bufs=2 * KT,
                                        name=f"x16_{l}_{d2}_{ci}")
                        ps = psB.tile([128, 400], f32, tag="ps",
                                         name=f"pso{l}_{d2}_{ci}")
                        for kt in range(NCTX):
                            nc.tensor.matmul(
                                ps[:], ow_t[:, kt, d2 * 128:(d2 + 1) * 128],
                                ctx_big[ci][:, kt, :],
                                start=(kt == 0), stop=(kt == NCTX - 1))
                        nc.vector.scalar_tensor_tensor(
                            xt[:], ps[:], obe_t[:, d2:d2 + 1], h16[(d2, ci)][:],
                            op0=ALU.add, op1=ALU.add)
                        x16.append(xt)
                    return layer_norm(x16, ci, 0, bcol_t, grow_t, l, "ln1")

                def tail2(ci, o1c, last):
                    """FFN + residual + LN2 for one chunk."""
                    mid = []
                    for m in range(NMID):
                        mt = apool.tile([128, 400], DT_MM, tag="mid", bufs=2 * NMID,
                                        name=f"mid{l}_{m}_{ci}")
                        ps = psB.tile([128, 400], f32, tag="ps",
                                         name=f"psf1{l}_{m}_{ci}")
                        for k in range(KT):
                            nc.tensor.matmul(
                                ps[:], ff1_t[:, k, m * 128:(m + 1) * 128],
                                o1c[k][:],
                                start=(k == 0), stop=(k == KT - 1))
                        if m % 2 == 0:
                            nc.scalar.activation(mt[:], ps[:], AF.Relu,
                                                 bias=f1b_t[:, m:m + 1])
                        else:
                            nc.vector.tensor_scalar(
                                mt[:], ps[:], f1b_t[:, m:m + 1], 0.0,
                                op0=ALU.add, op1=ALU.max)
                        mid.append(mt)
                    x2c = []
                    for d2 in range(KT):
                        xt = apool.tile([128, 400], DT_MM, tag="x2", bufs=2 * KT,
                                        name=f"x2_{l}_{d2}_{ci}")
                        ps = psB.tile([128, 400], f32, tag="ps",
                                         name=f"psf2{l}_{d2}_{ci}")
                        for kt in range(NMID):
                            nc.tensor.matmul(
                                ps[:], ff2_t[:, kt, d2 * 128:(d2 + 1) * 128],
                                mid[kt][:],
                                start=(kt == 0), stop=(kt == NMID - 1))
                        nc.vector.scalar_tensor_tensor(
                            xt[:], ps[:], f2b_t[:, d2:d2 + 1], o1c[d2][:],
                            op0=ALU.add, op1=ALU.add)
                        x2c.append(xt)
                    return layer_norm(x2c, ci, 1, bcol_t, grow_t, l, "ln2",
                                      last=last)

                # staggered emission: attention blocks fill LN-chain gaps
                last = (l == L_RUN - 1)
                att_seq(0)
                att_seq(1)
                o1_0 = tail1(0)
                att_seq(2)
                h_0 = tail2(0, o1_0, last)
                att_seq(3)
                o1_1 = tail1(1)
                h_1 = tail2(1, o1_1, last)
                newh = {}
                for k in range(KT):
                    newh[(k, 0)] = h_0[k]
                    newh[(k, 1)] = h_1[k]
                h16 = newh

    nc.compile()
    return nc


def _fold_weights(wqkv_w, wqkv_b, A1, A2, A3, A4, tnb, out_w, out_b):
    """Fold the TN contraction into dense weights; fold v-bias into out bias;
    fold 1/sqrt(D) into Q."""
    wqkv_w = np.asarray(wqkv_w, np.float32)
    wqkv_b = np.asarray(wqkv_b, np.float32)
    out_w = np.asarray(out_w, np.float32)
    out_b = np.asarray(out_b, np.float32)
    tnb = np.asarray(tnb, np.float32)
    scale = 1.0 / np.sqrt(np.float32(D))

    W_full = np.zeros((L, 3, D, H * D), np.float32)
    b_full = np.zeros((L, 3, H * D), np.float32)
    for l in range(L):
        for x in range(3):
            wt = np.einsum('pmi,qmnj,rnok,tol->pqrtijkl',
                           np.asarray(A1[l, x], np.float64),
                           np.asarray(A2[l, x], np.float64),
                           np.asarray(A3[l, x], np.float64),
                           np.asarray(A4[l, x], np.float64),
                           optimize=True).reshape(D, 4 * D).astype(np.float32)
            W_full[l, x] = np.concatenate([wqkv_w[l, x], wt], axis=1)
            b_full[l, x] = np.concatenate([wqkv_b[l, x], tnb[l, x]])
    W_full[:, 0] *= scale
    b_full[:, 0] *= scale

    wqk = np.concatenate([W_full[:, 0], W_full[:, 1]], axis=2)   # [L, 256, 3072]
    bqk = np.concatenate([b_full[:, 0], b_full[:, 1]], axis=1)   # [L, 3072]
    wv = W_full[:, 2]                                            # [L, 256, 1536]
    bv = b_full[:, 2]                                            # [L, 1536]
    obe = out_b + np.einsum('lc,lcd->ld', bv, out_w)             # [L, 256]
    return wqk, bqk, wv, obe


def _pack_w(x, nk):
    """[L, nk*128, M] -> [L, 128, nk, M] (partition-major SBUF layout)."""
    Lh, K, M = x.shape
    return np.ascontiguousarray(
        x.reshape(Lh, nk, 128, M).transpose(0, 2, 1, 3))


def _pack_cols(x, n):
    """[L, n*128] -> [L, 128, n]."""
    return np.ascontiguousarray(x.reshape(L, n, 128).transpose(0, 2, 1))


def kernel(**inputs):
    tokens = np.asarray(inputs["tokens"])
    tok_emb = np.asarray(inputs["tok_emb"], np.float32)
    pos_emb = np.asarray(inputs["pos_emb"], np.float32)

    wqk, bqk, wv, obe = _fold_weights(
        inputs["wqkv_w"], inputs["wqkv_b"], inputs["A1"], inputs["A2"],
        inputs["A3"], inputs["A4"], inputs["tnb"], inputs["out_w"],
        inputs["out_b"])
    ff1 = np.asarray(inputs["ff1_w"], np.float32)
    f1b = np.asarray(inputs["ff1_b"], np.float32)
    ff2 = np.asarray(inputs["ff2_w"], np.float32)
    f2b = np.asarray(inputs["ff2_b"], np.float32)
    ow = np.asarray(inputs["out_w"], np.float32)

    # LN biases as per-partition cols [L,128,2KT]; gains as fp16 rows
    bcol = np.stack([np.asarray(inputs["ln1_b"], np.float32),
                     np.asarray(inputs["ln2_b"], np.float32)], axis=1)  # [L,2,256]
    bcol = np.ascontiguousarray(
        bcol.reshape(L, 2, KT, 128).transpose(0, 3, 1, 2).reshape(L, 128, 2 * KT))
    grow = np.stack([np.asarray(inputs["ln1_g"], np.float32),
                     np.asarray(inputs["ln2_g"], np.float32)], axis=1)  # [L,2,256]
    grow = np.ascontiguousarray(grow.reshape(L, 1, 2 * KT * 128)).astype(NP_MM)

    shared = {
        "wqk": _pack_w(wqk.astype(NP_MM), KT),
        "bqk": _pack_cols(bqk, NQK),
        "wv": _pack_w(wv.astype(NP_MM), KT),
        "obe": _pack_cols(obe, KT),
        "ow": _pack_w(ow.astype(NP_MM), NCTX),
        "ff1": _pack_w(ff1.astype(NP_MM), KT),
        "f1b": _pack_cols(f1b, NMID),
        "ff2": _pack_w(ff2.astype(NP_MM), NMID),
        "f2b": _pack_cols(f2b, KT),
        "bcol": bcol,
        "grow": grow,
    }

    h0 = tok_emb[tokens] + pos_emb[None]          # [B, S, D] f32
    maskbias = np.where(tokens == 0, np.float32(-1e9),
                        np.float32(0.0)) - np.float32(CSHIFT)   # [B,S]

    in_maps = []
    for c in range(N_CORES):
        hc = h0[c * BS:(c + 1) * BS].reshape(T, D)
        hdim = np.ascontiguousarray(hc.T.reshape(KT, 128, T)).astype(NP_MM)
        mc = np.full((128, BS * KT), -1e9, np.float32)
        for b in range(BS):
            mb = maskbias[c * BS + b]             # [S]
            mc[0:128, b * KT + 0] = mb[0:128]
            mc[0:72, b * KT + 1] = mb[128:200]
        m = dict(shared)
        m["h0"] = hdim
        m["maskc"] = np.ascontiguousarray(mc)
        in_maps.append(m)

    if "nc" not in _CACHE:
        _CACHE["nc"] = _build_program()
    nc = _CACHE["nc"]
    _CACHE["in_maps"] = in_maps

    res = run_bass_kernel_spmd(nc, in_maps, list(range(N_CORES)))
    outs = []
    for c in range(N_CORES):
        od = res.results[c]["out"].reshape(D, T)      # dim-major
        outs.append(od.T.reshape(BS, S, D))
    return np.concatenate(outs, axis=0).astype(np.float32)


if __name__ == "__main__":
    import reference
    inputs = {k: np.asarray(v) for k, v in reference.setup_inputs().items()}
    got = kernel(**inputs)
    exp = np.asarray(reference.reference(**inputs))
    err = np.abs(got - exp).max() / np.abs(exp).max()
    print(f"Relative error: {err:.3e}")


# revision 17
# speedup vs baseline: 1.1777x; 1.0778x over previous
"""Trainium2 Bass kernel for nn_BERT_tensor (8-layer BERT with tensor-network heads).

Strategy (v3):
  - Data-parallel over batch: 32 seqs -> 4 seqs (800 tokens) per core x 8 cores.
  - Host folds the MPO tensor-network contraction (A1..A4) into a dense
    [256 -> 1024] weight per (layer, q/k/v); QKV is one dense matmul.
  - Everything stays DIM-MAJOR ([dim, token]); zero PE transposes:
    * attention computes scores TRANSPOSED ([kpos, qpos]); softmax
      max-subtraction replaced by a constant shift (scores bounded ~|14|);
      denominator via ones-matmul; division via reciprocal_approx_fast;
      normalization fused into ctx evacuation.
    * LayerNorm dim-major: stats via ones-matmuls over the partition axis;
      rstd = exp(-0.5*ln(var+eps)) on ScalarE (no Sqrt table set, no slow
      DVE reciprocal); per-token scale/offset broadcast via rank-1
      outer-product matmuls with the LN gain as the stationary operand.
  - fp16 matmul inputs everywhere (fp32 PSUM accumulation).
  - Single 1-bank PSUM pool (8 slots), 400-col chunks for fine-grained
    pipelining to keep TensorE dense (HAM warm).
"""
import numpy as np
from contextlib import ExitStack

import concourse.bass as bass
import concourse.bacc as bacc
import concourse.tile as tile
import concourse.mybir as mybir
from concourse.bass_utils import run_bass_kernel_spmd

dt = mybir.dt
AF = mybir.ActivationFunctionType
ALU = mybir.AluOpType

# problem constants (hardcoded per contract)
B, S, D = 32, 200, 256
H, DFF, VOCAB, L, TD = 6, 1024, 3500, 8, 2
N_CORES = 8
BS = B // N_CORES            # 4 seqs per core
T = BS * S                   # 800 tokens per core
KT = D // 128                # 2 k-tiles over emb dim
NQK = (2 * H * D) // 128     # 24 m-tiles over Q|K outdim (3072)
NCTX = (H * D) // 128        # 12 tiles over ctx dim (1536)
NMID = DFF // 128            # 8 tiles over ffn hidden
MMCH = [(0, 400), (400, 400)]       # N chunks of T (each fits one PSUM bank)
SEQ_TILES = [(0, 128), (128, 72)]   # per-seq kpos tiles
EPS = 1e-6
CSHIFT = 6.0                 # softmax constant shift (max |score| ~ 14)

import os
L_RUN = int(os.environ.get("BERT_L_RUN", str(L)))
DT_MM = dt.float16
NP_MM = np.float16

_CACHE = {}


def _patch_act_tables():
    """Force every activation function to resolve to the
    natural_log_exp_and_others table set (it contains exp/ln/square/
    identity/copy/relu — everything this kernel uses), so exactly one
    ACT_TABLE_LOAD is emitted instead of one per exp<->ln alternation.
    Set names and their act_info.json indices are preserved."""
    import concourse.hw_specs as hw_specs
    import concourse.bacc as bacc_mod
    if getattr(bacc_mod, "_act_tables_patched", False):
        return
    orig = hw_specs.get_activation_tables

    def only_nle(arch):
        t = orig(arch)
        return {k: (v if k == "natural_log_exp_and_others" else set())
                for k, v in t.items()}

    bacc_mod.get_activation_tables = only_nle
    bacc_mod._act_tables_patched = True


def _build_program():
    _patch_act_tables()
    nc = bacc.Bacc("TRN2", target_bir_lowering=False, debug=False,
                   num_devices=N_CORES)

    f32 = dt.float32
    inp = {}

    def din(name, shape, dty):
        inp[name] = nc.dram_tensor(name, list(shape), dty, kind="ExternalInput").ap()
        return inp[name]

    h0_d = din("h0", [KT, 128, T], DT_MM)
    maskc_d = din("maskc", [128, BS * KT], f32)        # exp bias cols (mask - C)
    wqk_d = din("wqk", [L, 128, KT, 2 * H * D], DT_MM)
    bqk_d = din("bqk", [L, 128, NQK], f32)
    wv_d = din("wv", [L, 128, KT, H * D], DT_MM)
    ow_d = din("ow", [L, 128, NCTX, D], DT_MM)
    obe_d = din("obe", [L, 128, KT], f32)
    ff1_d = din("ff1", [L, 128, KT, DFF], DT_MM)
    f1b_d = din("f1b", [L, 128, NMID], f32)
    ff2_d = din("ff2", [L, 128, NMID, D], DT_MM)
    f2b_d = din("f2b", [L, 128, KT], f32)
    bcol_d = din("bcol", [L, 128, 2 * KT], f32)        # b1|b2 cols
    grow_d = din("grow", [L, 1, 2 * KT * 128], DT_MM)  # g1|g2 rows (fp16)
    out_d = nc.dram_tensor("out", [KT, 128, T], f32, kind="ExternalOutput").ap()

    with tile.TileContext(nc) as tc:
        with ExitStack() as ctx:
            cpool = ctx.enter_context(tc.tile_pool(name="const", bufs=1))
            wpool = ctx.enter_context(tc.tile_pool(name="weights", bufs=1))
            apool = ctx.enter_context(tc.tile_pool(name="acts", bufs=1))
            spool = ctx.enter_context(tc.tile_pool(name="scratch", bufs=1))
            psA = ctx.enter_context(tc.tile_pool(name="psA", bufs=4, space="PSUM"))
            psB = ctx.enter_context(tc.tile_pool(name="psB", bufs=2, space="PSUM"))
            psC = ctx.enter_context(tc.tile_pool(name="psC", bufs=2, space="PSUM"))

            ones16 = cpool.tile([128, 128], DT_MM, tag="ones16", name="ones16")
            nc.vector.memset(ones16[:], 1.0)
            eps_t = cpool.tile([1, 1], f32, tag="eps", name="eps_t")
            nc.vector.memset(eps_t[:], EPS)
            maskc = cpool.tile([128, BS * KT], f32, tag="maskc", name="maskc")
            nc.sync.dma_start(maskc[:], maskc_d[:])

            # initial h: per (k, ci) fp16 [128, 400] tiles (dim-major)
            h16 = {}
            for k in range(KT):
                for ci, (co, cs) in enumerate(MMCH):
                    t = apool.tile([128, 400], DT_MM, tag="h16", bufs=2 * KT,
                                   name=f"h16_0_{k}_{ci}")
                    nc.sync.dma_start(t[:], h0_d[k, :, co:co + cs])
                    h16[(k, ci)] = t

            def layer_norm(x16, ci, ln_i, bcol_t, grow_t, l, tag, last=False):
                """Dim-major LN of one token-chunk (KT fp16 [128,400] tiles)."""
                sq = []
                for k in range(KT):
                    s = spool.tile([128, 400], DT_MM, tag="sq", bufs=2 * KT,
                                   name=f"{tag}sq{l}_{ci}_{k}")
                    nc.scalar.activation(s[:], x16[k][:], AF.Square)
                    sq.append(s)
                outs = [apool.tile([128, 400], DT_MM, tag=f"{tag}o", bufs=4 * KT,
                                   name=f"{tag}o{l}_{ci}_{k}") for k in range(KT)]
                ssum = psC.tile([1, 400], f32, tag="ps", name=f"{tag}ss{l}_{ci}")
                for k in range(KT):
                    nc.tensor.matmul(ssum[:], ones16[:, 0:1], x16[k][:],
                                     start=(k == 0), stop=(k == KT - 1))
                ssq = psC.tile([1, 400], f32, tag="ps", name=f"{tag}sk{l}_{ci}")
                for k in range(KT):
                    nc.tensor.matmul(ssq[:], ones16[:, 0:1], sq[k][:],
                                     start=(k == 0), stop=(k == KT - 1))
                nm = spool.tile([1, 400], f32, tag="lnrow", bufs=6,
                                name=f"{tag}nm{l}_{ci}")
                nc.vector.tensor_scalar_mul(nm[:], ssum[:], -1.0 / D)
                msq = spool.tile([1, 400], f32, tag="lnrow", bufs=6,
                                 name=f"{tag}ms{l}_{ci}")
                nc.scalar.activation(msq[:], ssum[:], AF.Square, scale=-1.0 / D)
                var = spool.tile([1, 400], f32, tag="lnrow", bufs=6,
                                 name=f"{tag}va{l}_{ci}")
                nc.vector.scalar_tensor_tensor(
                    var[:], ssq[:], 1.0 / D, msq[:], op0=ALU.mult,
                    op1=ALU.subtract)
                lnv = spool.tile([1, 400], f32, tag="lnrow", bufs=6,
                                 name=f"{tag}lv{l}_{ci}")
                nc.scalar.activation(lnv[:], var[:], AF.Ln, bias=eps_t[:])
                rstd = spool.tile([1, 400], DT_MM, tag="lnrow16", bufs=4,
                                  name=f"{tag}rs{l}_{ci}")
                nc.scalar.activation(rstd[:], lnv[:], AF.Exp, scale=-0.5)
                mr = spool.tile([1, 400], DT_MM, tag="lnrow16", bufs=4,
                                name=f"{tag}mr{l}_{ci}")
                nc.vector.tensor_tensor(mr[:], nm[:], rstd[:], op=ALU.mult)
                for k in range(KT):
                    gsl = grow_t[0:1, (ln_i * KT + k) * 128:(ln_i * KT + k + 1) * 128]
                    rbg = psC.tile([128, 400], f32, tag="ps",
                                      name=f"{tag}rb{l}_{ci}_{k}")
                    nc.tensor.matmul(rbg[:], gsl, rstd[:], start=True, stop=True)
                    mbg = psC.tile([128, 400], f32, tag="ps",
                                      name=f"{tag}mb{l}_{ci}_{k}")
                    nc.tensor.matmul(mbg[:], gsl, mr[:], start=True, stop=True)
                    u = spool.tile([128, 400], f32, tag="uv", bufs=3,
                                   name=f"{tag}u{l}_{ci}_{k}")
                    nc.vector.tensor_tensor(u[:], x16[k][:], rbg[:], op=ALU.mult)
                    bsl = bcol_t[:, ln_i * KT + k:ln_i * KT + k + 1]
                    nc.vector.scalar_tensor_tensor(
                        outs[k][:], u[:], bsl, mbg[:], op0=ALU.add, op1=ALU.add)
                    if last:
                        outf = spool.tile([128, 400], f32, tag="outf", bufs=KT,
                                          name=f"outf_{ci}_{k}")
                        nc.vector.scalar_tensor_tensor(
                            outf[:], u[:], bsl, mbg[:], op0=ALU.add, op1=ALU.add)
                        co = MMCH[ci][0]
                        nc.sync.dma_start(out_d[k, :, co:co + 400], outf[:])
                return outs

            for l in range(L_RUN):
                # ---- layer weights ----
                if l == 0:
                    wqk_t = wpool.tile([128, KT, 2 * H * D], DT_MM, tag="wqk",
                                       bufs=1, name=f"wqk{l}")
                    nc.sync.dma_start(wqk_t[:], wqk_d[l])
                wv_t = wpool.tile([128, KT, H * D], DT_MM, tag="wv", bufs=2,
                                  name=f"wv{l}")
                nc.sync.dma_start(wv_t[:], wv_d[l])
                ow_t = wpool.tile([128, NCTX, D], DT_MM, tag="ow", bufs=2,
                                  name=f"ow{l}")
                nc.sync.dma_start(ow_t[:], ow_d[l])
                ff1_t = wpool.tile([128, KT, DFF], DT_MM, tag="ff1", bufs=2,
                                   name=f"ff1{l}")
                nc.sync.dma_start(ff1_t[:], ff1_d[l])
                ff2_t = wpool.tile([128, NMID, D], DT_MM, tag="ff2", bufs=2,
                                   name=f"ff2{l}")
                nc.sync.dma_start(ff2_t[:], ff2_d[l])
                if l == 0:
                    bqk_t = wpool.tile([128, NQK], f32, tag="bqk", bufs=2,
                                       name=f"bqk{l}")
                    nc.sync.dma_start(bqk_t[:], bqk_d[l])
                obe_t = wpool.tile([128, KT], f32, tag="obe", bufs=2, name=f"obe{l}")
                nc.sync.dma_start(obe_t[:], obe_d[l])
                f1b_t = wpool.tile([128, NMID], f32, tag="f1b", bufs=2, name=f"f1b{l}")
                nc.sync.dma_start(f1b_t[:], f1b_d[l])
                f2b_t = wpool.tile([128, KT], f32, tag="f2b", bufs=2, name=f"f2b{l}")
                nc.sync.dma_start(f2b_t[:], f2b_d[l])
                bcol_t = wpool.tile([128, 2 * KT], f32, tag="bcol", bufs=2,
                                    name=f"bcol{l}")
                nc.sync.dma_start(bcol_t[:], bcol_d[l])
                grow_t = wpool.tile([1, 2 * KT * 128], DT_MM, tag="grow", bufs=2,
                                    name=f"grow{l}")
                nc.sync.dma_start(grow_t[:], grow_d[l])

                # ---- QK: dim-major fp16, per (m, ci) [128,400] tiles ----
                def qk_phase(lq, wqk_q, bqk_q, h16_q, ci):
                    out = {}
                    for m in range(NQK):
                        qt = apool.tile([128, 400], DT_MM, tag="qk", bufs=2 * NQK,
                                        name=f"qk{lq}_{m}_{ci}")
                        ps = psB.tile([128, 400], f32, tag="ps",
                                      name=f"psqk{lq}_{m}_{ci}")
                        for k in range(KT):
                            nc.tensor.matmul(
                                ps[:], wqk_q[:, k, m * 128:(m + 1) * 128],
                                h16_q[(k, ci)][:],
                                start=(k == 0), stop=(k == KT - 1))
                        if m % 2 == 0:
                            nc.scalar.activation(qt[:], ps[:], AF.Identity,
                                                 bias=bqk_q[:, m:m + 1])
                        else:
                            nc.vector.tensor_scalar_add(qt[:], ps[:],
                                                        bqk_q[:, m:m + 1])
                        out[(m, ci)] = qt
                    return out

                if l == 0:
                    qk = {}
                    for ci in range(2):
                        qk.update(qk_phase(0, wqk_t, bqk_t, h16, ci))
                else:
                    qk = qk_carry

                ctx_big = {ci: apool.tile([128, NCTX, 400], DT_MM, tag="ctxb",
                                          bufs=2, name=f"ctxb{l}_{ci}")
                           for ci in range(2)}

                def att_seq(b):
                    """V + attention for one sequence (transpose-free)."""
                    ci, sb = b // 2, (b % 2) * S
                    vt = {}
                    for ti, (to, ts) in enumerate(SEQ_TILES):
                        v = apool.tile([128, H * D], DT_MM, tag="v", bufs=2 * BS,
                                       name=f"v{l}_{b}_{ti}")
                        for nch in range(3):
                            ps = psA.tile([128, 512], f32, tag="ps",
                                             name=f"psv{l}_{b}_{ti}_{nch}")
                            for k in range(KT):
                                nc.tensor.matmul(
                                    ps[0:ts, :],
                                    h16[(k, ci)][:, sb + to:sb + to + ts],
                                    wv_t[:, k, nch * 512:(nch + 1) * 512],
                                    start=(k == 0), stop=(k == KT - 1))
                            nc.scalar.activation(
                                v[0:ts, nch * 512:(nch + 1) * 512], ps[0:ts, :],
                                AF.Copy)
                        vt[ti] = v
                    for h in range(H):
                        sps = psA.tile([128, 400], f32, tag="ps",
                                          name=f"sps{l}_{b}_{h}")
                        for ti, (to, ts) in enumerate(SEQ_TILES):
                            for k in range(KT):
                                nc.tensor.matmul(
                                    sps[0:ts, ti * S:(ti + 1) * S],
                                    qk[((H + h) * KT + k, ci)][:, sb + to:sb + to + ts],
                                    qk[(h * KT + k, ci)][:, sb:sb + S],
                                    start=(k == 0), stop=(k == KT - 1))
                        at = spool.tile([128, 400], DT_MM, tag="attn", bufs=6,
                                        name=f"at{l}_{b}_{h}")
                        for ti, (to, ts) in enumerate(SEQ_TILES):
                            nc.scalar.activation(
                                at[0:ts, ti * S:(ti + 1) * S],
                                sps[0:ts, ti * S:(ti + 1) * S], AF.Exp,
                                bias=maskc[0:ts, b * KT + ti:b * KT + ti + 1])
                        dps = psA.tile([128, S], f32, tag="ps",
                                          name=f"dps{l}_{b}_{h}")
                        for ti, (to, ts) in enumerate(SEQ_TILES):
                            nc.tensor.matmul(dps[:], ones16[0:ts, :],
                                             at[0:ts, ti * S:(ti + 1) * S],
                                             start=(ti == 0), stop=(ti == 1))
                        rden = spool.tile([128, S], f32, tag="rden", bufs=3,
                                          name=f"rden{l}_{b}_{h}")
                        nc.vector.reciprocal_approx_fast(rden[:], dps[:])
                        cps = psA.tile([128, 400], f32, tag="ps",
                                          name=f"cps{l}_{b}_{h}")
                        for dvh in range(2):
                            for ti, (to, ts) in enumerate(SEQ_TILES):
                                nc.tensor.matmul(
                                    cps[:, dvh * S:(dvh + 1) * S],
                                    vt[ti][0:ts,
                                           h * D + dvh * 128:h * D + (dvh + 1) * 128],
                                    at[0:ts, ti * S:(ti + 1) * S],
                                    start=(ti == 0), stop=(ti == 1))
                        nc.vector.tensor_tensor(
                            ctx_big[ci][:, 2 * h:2 * h + 2, sb:sb + S],
                            cps[:].rearrange("p (v s) -> p v s", v=2),
                            rden[:].rearrange("p (o s) -> p o s", o=1)
                                   .broadcast_to([128, 2, S]),
                            op=ALU.mult)

                def tail1(ci):
                    """out-projection + residual + LN1 for one chunk."""
                    x16 = []
                    for d2 in range(KT):
                        xt = apool.tile([128, 400], DT_MM, tag="x16", bufs=2 * KT,
                                        name=f"x16_{l}_{d2}_{ci}")
                        ps = psB.tile([128, 400], f32, tag="ps",
                                         name=f"pso{l}_{d2}_{ci}")
                        for kt in range(NCTX):
                            nc.tensor.matmul(
                                ps[:], ow_t[:, kt, d2 * 128:(d2 + 1) * 128],
                                ctx_big[ci][:, kt, :],
                                start=(kt == 0), stop=(kt == NCTX - 1))
                        nc.vector.scalar_tensor_tensor(
                            xt[:], ps[:], obe_t[:, d2:d2 + 1], h16[(d2, ci)][:],
                            op0=ALU.add, op1=ALU.add)
                        x16.append(xt)
                    return layer_norm(x16, ci, 0, bcol_t, grow_t, l, "ln1")

                def tail2(ci, o1c, last):
                    """FFN + residual + LN2 for one chunk."""
                    mid = []
                    for m in range(NMID):
                        mt = apool.tile([128, 400], DT_MM, tag="mid", bufs=2 * NMID,
                                        name=f"mid{l}_{m}_{ci}")
                        ps = psB.tile([128, 400], f32, tag="ps",
                                         name=f"psf1{l}_{m}_{ci}")
                        for k in range(KT):
                            nc.tensor.matmul(
                                ps[:], ff1_t[:, k, m * 128:(m + 1) * 128],
                                o1c[k][:],
                                start=(k == 0), stop=(k == KT - 1))
                        if m % 2 == 0:
                            nc.scalar.activation(mt[:], ps[:], AF.Relu,
                                                 bias=f1b_t[:, m:m + 1])
                        else:
                            nc.vector.tensor_scalar(
                                mt[:], ps[:], f1b_t[:, m:m + 1], 0.0,
                                op0=ALU.add, op1=ALU.max)
                        mid.append(mt)
                    x2c = []
                    for d2 in range(KT):
                        xt = apool.tile([128, 400], DT_MM, tag="x2", bufs=2 * KT,
                                        name=f"x2_{l}_{d2}_{ci}")
                        ps = psB.tile([128, 400], f32, tag="ps",
                                         name=f"psf2{l}_{d2}_{ci}")
                        for kt in range(NMID):
                            nc.tensor.matmul(
                                ps[:], ff2_t[:, kt, d2 * 128:(d2 + 1) * 128],
                                mid[kt][:],
                                start=(kt == 0), stop=(kt == NMID - 1))
                        nc.vector.scalar_tensor_tensor(
                            xt[:], ps[:], f2b_t[:, d2:d2 + 1], o1c[d2][:],
                            op0=ALU.add, op1=ALU.add)
                        x2c.append(xt)
                    return layer_norm(x2c, ci, 1, bcol_t, grow_t, l, "ln2",
                                      last=last)

                # staggered emission: attention blocks + next-layer QK
                # fill the LN-chain windows
                last = (l == L_RUN - 1)
                att_seq(0)
                att_seq(1)
                o1_0 = tail1(0)
                att_seq(2)
                h_0 = tail2(0, o1_0, last)
                att_seq(3)
                o1_1 = tail1(1)
                h16n = {}
                if not last:
                    wqk_n = wpool.tile([128, KT, 2 * H * D], DT_MM, tag="wqk",
                                       bufs=1, name=f"wqk{l + 1}")
                    nc.sync.dma_start(wqk_n[:], wqk_d[l + 1])
                    bqk_n = wpool.tile([128, NQK], f32, tag="bqk", bufs=2,
                                       name=f"bqk{l + 1}")
                    nc.sync.dma_start(bqk_n[:], bqk_d[l + 1])
                    for k in range(KT):
                        h16n[(k, 0)] = h_0[k]
                    qk_carry = qk_phase(l + 1, wqk_n, bqk_n, h16n, 0)
                h_1 = tail2(1, o1_1, last)
                if not last:
                    for k in range(KT):
                        h16n[(k, 1)] = h_1[k]
                    qk_carry.update(qk_phase(l + 1, wqk_n, bqk_n, h16n, 1))
                newh = {}
                for k in range(KT):
                    newh[(k, 0)] = h_0[k]
                    newh[(k, 1)] = h_1[k]
                h16 = newh

    nc.compile()
    return nc


def _fold_weights(wqkv_w, wqkv_b, A1, A2, A3, A4, tnb, out_w, out_b):
    """Fold the TN contraction into dense weights; fold v-bias into out bias;
    fold 1/sqrt(D) into Q."""
    wqkv_w = np.asarray(wqkv_w, np.float32)
    wqkv_b = np.asarray(wqkv_b, np.float32)
    out_w = np.asarray(out_w, np.float32)
    out_b = np.asarray(out_b, np.float32)
    tnb = np.asarray(tnb, np.float32)
    scale = 1.0 / np.sqrt(np.float32(D))

    W_full = np.zeros((L, 3, D, H * D), np.float32)
    b_full = np.zeros((L, 3, H * D), np.float32)
    for l in range(L):
        for x in range(3):
            wt = np.einsum('pmi,qmnj,rnok,tol->pqrtijkl',
                           np.asarray(A1[l, x], np.float64),
                           np.asarray(A2[l, x], np.float64),
                           np.asarray(A3[l, x], np.float64),
                           np.asarray(A4[l, x], np.float64),
                           optimize=True).reshape(D, 4 * D).astype(np.float32)
            W_full[l, x] = np.concatenate([wqkv_w[l, x], wt], axis=1)
            b_full[l, x] = np.concatenate([wqkv_b[l, x], tnb[l, x]])
    W_full[:, 0] *= scale
    b_full[:, 0] *= scale

    wqk = np.concatenate([W_full[:, 0], W_full[:, 1]], axis=2)   # [L, 256, 3072]
    bqk = np.concatenate([b_full[:, 0], b_full[:, 1]], axis=1)   # [L, 3072]
    wv = W_full[:, 2]                                            # [L, 256, 1536]
    bv = b_full[:, 2]                                            # [L, 1536]
    obe = out_b + np.einsum('lc,lcd->ld', bv, out_w)             # [L, 256]
    return wqk, bqk, wv, obe


def _pack_w(x, nk):
    """[L, nk*128, M] -> [L, 128, nk, M] (partition-major SBUF layout)."""
    Lh, K, M = x.shape
    return np.ascontiguousarray(
        x.reshape(Lh, nk, 128, M).transpose(0, 2, 1, 3))


def _pack_cols(x, n):
    """[L, n*128] -> [L, 128, n]."""
    return np.ascontiguousarray(x.reshape(L, n, 128).transpose(0, 2, 1))


def kernel(**inputs):
    tokens = np.asarray(inputs["tokens"])
    tok_emb = np.asarray(inputs["tok_emb"], np.float32)
    pos_emb = np.asarray(inputs["pos_emb"], np.float32)

    wqk, bqk, wv, obe = _fold_weights(
        inputs["wqkv_w"], inputs["wqkv_b"], inputs["A1"], inputs["A2"],
        inputs["A3"], inputs["A4"], inputs["tnb"], inputs["out_w"],
        inputs["out_b"])
    ff1 = np.asarray(inputs["ff1_w"], np.float32)
    f1b = np.asarray(inputs["ff1_b"], np.float32)
    ff2 = np.asarray(inputs["ff2_w"], np.float32)
    f2b = np.asarray(inputs["ff2_b"], np.float32)
    ow = np.asarray(inputs["out_w"], np.float32)

    # LN biases as per-partition cols [L,128,2KT]; gains as fp16 rows
    bcol = np.stack([np.asarray(inputs["ln1_b"], np.float32),
                     np.asarray(inputs["ln2_b"], np.float32)], axis=1)  # [L,2,256]
    bcol = np.ascontiguousarray(
        bcol.reshape(L, 2, KT, 128).transpose(0, 3, 1, 2).reshape(L, 128, 2 * KT))
    grow = np.stack([np.asarray(inputs["ln1_g"], np.float32),
                     np.asarray(inputs["ln2_g"], np.float32)], axis=1)  # [L,2,256]
    grow = np.ascontiguousarray(grow.reshape(L, 1, 2 * KT * 128)).astype(NP_MM)

    shared = {
        "wqk": _pack_w(wqk.astype(NP_MM), KT),
        "bqk": _pack_cols(bqk, NQK),
        "wv": _pack_w(wv.astype(NP_MM), KT),
        "obe": _pack_cols(obe, KT),
        "ow": _pack_w(ow.astype(NP_MM), NCTX),
        "ff1": _pack_w(ff1.astype(NP_MM), KT),
        "f1b": _pack_cols(f1b, NMID),
        "ff2": _pack_w(ff2.astype(NP_MM), NMID),
        "f2b": _pack_cols(f2b, KT),
        "bcol": bcol,
        "grow": grow,
    }

    h0 = tok_emb[tokens] + pos_emb[None]          # [B, S, D] f32
    maskbias = np.where(tokens == 0, np.float32(-1e9),
                        np.float32(0.0)) - np.float32(CSHIFT)   # [B,S]

    in_maps = []
    for c in range(N_CORES):
        hc = h0[c * BS:(c + 1) * BS].reshape(T, D)
        hdim = np.ascontiguousarray(hc.T.reshape(KT, 128, T)).astype(NP_MM)
        mc = np.full((128, BS * KT), -1e9, np.float32)
        for b in range(BS):
            mb = maskbias[c * BS + b]             # [S]
            mc[0:128, b * KT + 0] = mb[0:128]
            mc[0:72, b * KT + 1] = mb[128:200]
        m = dict(shared)
        m["h0"] = hdim
        m["maskc"] = np.ascontiguousarray(mc)
        in_maps.append(m)

    if "nc" not in _CACHE:
        _CACHE["nc"] = _build_program()
    nc = _CACHE["nc"]
    _CACHE["in_maps"] = in_maps

    res = run_bass_kernel_spmd(nc, in_maps, list(range(N_CORES)))
    outs = []
    for c in range(N_CORES):
        od = res.results[c]["out"].reshape(D, T)      # dim-major
        outs.append(od.T.reshape(BS, S, D))
    return np.concatenate(outs, axis=0).astype(np.float32)


if __name__ == "__main__":
    import reference
    inputs = {k: np.asarray(v) for k, v in reference.setup_inputs().items()}
    got = kernel(**inputs)
    exp = np.asarray(reference.reference(**inputs))
    err = np.abs(got - exp).max() / np.abs(exp).max()
    print(f"Relative error: {err:.3e}")
